# revision 43
# baseline (speedup 1.0000x reference)
"""LeNet C3 grouped-conv layer as a Trainium2 Bass/Tile kernel.

Math: y[b,o,h,w] = sum_{c,dy,dx} W[o,c,dy,dx] * x[b,c,h+dy,w+dx] + bias[o]
with W the dense 16x6x5x5 weight built from the C3 connectivity tables
(absent connections are zero).

Mapping (per core, 16 images of the batch):
  - Input tile: 20 consecutive rows x 6 channels of 4 images, partitions
    p = c*20 + dr (c-major; natural HBM order -> one 2D/3D 120 KB DMA per
    image per block), cols = img*256 + w.  K = 120.
  - Each 20-row block yields 16 output rows via TWO window phases: phase
    ph covers output rows h0+8*ph .. h0+8*ph+7.  Phase selection happens
    in the stationary lhsT: lhsT_ph[(c,dr), (o,s)] = W[o, c, dr-8*ph-s, dx]
    (zero outside the band).  Matmul cost is N-bound, so the K=120 zero
    padding is free.
  - 5 matmuls (dx = 0..4) accumulate in PSUM with the moving AP shifted by
    dx columns; images are processed in pairs: N = 508 (>=256 keeps
    float32r at 1 cycle/column).  Out cols 0..251 img A, 256..507 img B,
    252..255 garbage seam (skipped on store).
  - Bias is added during the PSUM->SBUF copy (DVE tensor_scalar_add).
  - Blocks: h0 = 0,16,...,224, then a final block at h0 = 236 (rows
    236..255) producing rows 236..251; rows 236..239 would duplicate
    block 14's output, so that phase stores only s = 4..7 (rows 240..243).
  - float32r (e8m11) inputs, pre-rounded on the host.
"""

import sys

sys.path.insert(0, "/opt/trn_rl_repo")

import numpy as np

_CH3 = np.array([[0, 1, 2], [1, 2, 3], [2, 3, 4], [3, 4, 5], [0, 4, 5], [0, 1, 5]])
_CH4 = np.array(
    [
        [0, 1, 2, 3],
        [1, 2, 3, 4],
        [2, 3, 4, 5],
        [0, 3, 4, 5],
        [0, 1, 4, 5],
        [0, 1, 2, 5],
        [0, 1, 3, 4],
        [1, 2, 4, 5],
        [0, 2, 3, 5],
    ]
)
_CH6 = np.array([[0, 1, 2, 3, 4, 5]])

_B_PER_CORE = 16  # 128 batch / 8 cores
_N_CORES = 8
_H = 256
_W = 256
_HO = 252
_WO = 252
_R = 20  # input rows per block
_K = 6 * _R  # 120 contraction partitions

_module_cache = {}


def _round_f32r(a):
    """Round fp32 to fp32r (e8m11): RNE into 11 mantissa bits, low 12 zero."""
    b = np.ascontiguousarray(a, dtype=np.float32).view(np.uint32)
    lsb = (b >> np.uint32(12)) & np.uint32(1)
    rnd = np.uint32((1 << 11) - 1) + lsb
    b2 = (b + rnd) & np.uint32(0xFFFFF000)
    return b2.view(np.float32)


def _round_in(a):
    """Convert a matmul input to the on-device dtype (fp32r or bf16)."""
    import os

    if os.environ.get("C3_BF16", "0") == "1":
        import ml_dtypes

        return np.ascontiguousarray(
            np.asarray(a, np.float32).astype(ml_dtypes.bfloat16)
        )
    return _round_f32r(a)


def _dense_weights(w3, b3, w4, b4, w6, b6):
    W = np.zeros((16, 6, 5, 5), np.float32)
    bias = np.zeros((16,), np.float32)
    for i in range(6):
        W[i, _CH3[i]] = w3[i]
    bias[0:6] = b3
    for i in range(9):
        W[6 + i, _CH4[i]] = w4[i]
    bias[6:15] = b4
    W[15, _CH6[0]] = w6[0]
    bias[15] = np.asarray(b6).reshape(-1)[0]
    return W, bias


def _host_tensors(w3, b3, w4, b4, w6, b6):
    W, bias = _dense_weights(w3, b3, w4, b4, w6, b6)
    # lhsT[(c, dr), (ph, dx, o, s)] = W[o, c, dr - 8*ph - s, dx]
    lhsT = np.zeros((6, _R, 2, 5, 16, 8), np.float32)
    for dr in range(_R):
        for ph in range(2):
            for s in range(8):
                dy = dr - 8 * ph - s
                if 0 <= dy < 5:
                    # [c, dx, o] <- W[o, c, dy, dx]
                    lhsT[:, dr, ph, :, :, s] = W[:, :, dy, :].transpose(1, 2, 0)
    lhsT = np.ascontiguousarray(lhsT.reshape(_K, 2 * 5 * 128))
    # final-block phase-0 variant: output rows h0+4+s (s = 0..3), compact
    # M = 64 with p = o*4 + s.
    lhsTp = np.zeros((6, _R, 5, 16, 4), np.float32)
    for dr in range(_R):
        for s in range(4):
            dy = dr - 4 - s
            if 0 <= dy < 5:
                lhsTp[:, dr, :, :, s] = W[:, :, dy, :].transpose(1, 2, 0)
    lhsTp = np.ascontiguousarray(lhsTp.reshape(_K, 320))
    biasf = np.repeat(bias, 8).reshape(128, 1).astype(np.float32)  # p = o*8+s
    biasp = np.repeat(bias, 4).reshape(64, 1).astype(np.float32)  # p = o*4+s
    return lhsT, lhsTp, biasf, biasp


import os

_USE_BF16 = os.environ.get("C3_BF16", "0") == "1"


def _build_module(reps=1):
    if ("nc", reps) in _module_cache:
        return _module_cache[("nc", reps)]

    import concourse.bacc as bacc
    import concourse.mybir as mybir
    from concourse.tile import TileContext

    f32 = mybir.dt.float32
    f32r = mybir.dt.bfloat16 if _USE_BF16 else mybir.dt.float32r

    nc = bacc.Bacc("TRN2", target_bir_lowering=False, debug=False)
    x = nc.dram_tensor("x", [_B_PER_CORE, 6, _H, _W], f32r, kind="ExternalInput").ap()
    lhsT = nc.dram_tensor("lhsT", [_K, 1280], f32r, kind="ExternalInput").ap()
    lhsTp = nc.dram_tensor("lhsTp", [_K, 320], f32r, kind="ExternalInput").ap()
    biasf = nc.dram_tensor("biasf", [128, 1], f32, kind="ExternalInput").ap()
    biasp = nc.dram_tensor("biasp", [64, 1], f32, kind="ExternalInput").ap()
    y = nc.dram_tensor(
        "y", [_B_PER_CORE, 16, _HO, _WO], f32, kind="ExternalOutput"
    ).ap()

    n_blk = 16  # 15 blocks at h0=16k + final block at h0=236

    with TileContext(nc) as tc:
        with (
            tc.tile_pool(name="const", bufs=1) as cpool,
            tc.tile_pool(name="xin", bufs=8) as xpool,
            tc.tile_pool(name="oup", bufs=6) as opool,
            tc.tile_pool(name="psum", bufs=8, space="PSUM") as ppool,
        ):
            wt = cpool.tile([_K, 1280], f32r)
            nc.sync.dma_start(out=wt, in_=lhsT)
            wtp = cpool.tile([_K, 320], f32r)
            nc.sync.dma_start(out=wtp, in_=lhsTp)
            bf = cpool.tile([128, 1], f32)
            nc.sync.dma_start(out=bf, in_=biasf)
            bp = cpool.tile([64, 1], f32)
            nc.sync.dma_start(out=bp, in_=biasp)

            out_ctr = 0
            for rep in range(reps):
              for quad in range(_B_PER_CORE // 4):
                bQ = 4 * quad
                for blk in range(n_blk):
                    h0 = 16 * blk if blk < n_blk - 1 else 236
                    xt = xpool.tile([_K, 1024], f32r)
                    for img in range(4):
                        # natural HBM order (c, h, w) pairs with partitions
                        # p = c*20 + dr; (h, w) rows are HBM-contiguous.
                        # Alternate the two HWDGE queues (SP / ACT).
                        ieng = nc.sync if img % 2 == 0 else nc.scalar
                        ieng.dma_start(
                            out=xt[:, img * 256 : (img + 1) * 256],
                            in_=x[bQ + img][:, h0 : h0 + _R, :],
                        )
                    # col = ph*1024 + img*256 + w
                    ot = opool.tile([128, 2048], f32)
                    final0 = blk == n_blk - 1
                    for ph in range(2):
                        M = 64 if (final0 and ph == 0) else 128
                        for g in range(2):
                            ps = ppool.tile([128, 508], f32)
                            for dx in range(5):
                                if M == 64:
                                    lw = wtp[:, dx * 64 : (dx + 1) * 64]
                                else:
                                    lw = wt[
                                        :,
                                        (ph * 5 + dx) * 128 : (ph * 5 + dx + 1) * 128,
                                    ]
                                nc.tensor.matmul(
                                    ps[0:M, :],
                                    lw,
                                    xt[:, 512 * g + dx : 512 * g + dx + 508],
                                    start=(dx == 0),
                                    stop=(dx == 4),
                                )
                            nc.vector.tensor_scalar_add(
                                ot[
                                    0:M,
                                    ph * 1024 + 512 * g : ph * 1024 + 512 * g + 508,
                                ],
                                ps[0:M, :],
                                bf if M == 128 else bp,
                            )
                    for img in range(4):
                        b = bQ + img
                        for ph in range(2):
                            col0 = ph * 1024 + img * 256
                            eng = nc.scalar if out_ctr % 2 == 0 else nc.sync
                            out_ctr += 1
                            if blk == n_blk - 1 and ph == 0:
                                # compact M=64 variant: rows 240..243 at
                                # partitions p = o*4 + s.
                                eng.dma_start(
                                    out=y[b][:, 240:244, :],
                                    in_=ot[0:64, col0 : col0 + 252],
                                )
                            else:
                                eng.dma_start(
                                    out=y[b][:, h0 + 8 * ph : h0 + 8 * ph + 8, :],
                                    in_=ot[:, col0 : col0 + 252],
                                )

    nc.compile()
    _module_cache[("nc", reps)] = nc
    return nc


def _run(inputs, trace=False):
    from concourse.bass_utils import run_bass_kernel_spmd

    x = _round_in(np.asarray(inputs["x"], dtype=np.float32))
    lhsT, lhsTp, biasf, biasp = _host_tensors(
        np.asarray(inputs["w3"], np.float32),
        np.asarray(inputs["b3"], np.float32),
        np.asarray(inputs["w4"], np.float32),
        np.asarray(inputs["b4"], np.float32),
        np.asarray(inputs["w6"], np.float32),
        np.asarray(inputs["b6"], np.float32),
    )
    lhsT = _round_in(lhsT)
    lhsTp = _round_in(lhsTp)
    nc = _build_module()
    in_maps = [
        {
            "x": np.ascontiguousarray(x[_B_PER_CORE * i : _B_PER_CORE * (i + 1)]),
            "lhsT": lhsT,
            "lhsTp": lhsTp,
            "biasf": biasf,
            "biasp": biasp,
        }
        for i in range(_N_CORES)
    ]
    res = run_bass_kernel_spmd(
        nc, in_maps, core_ids=list(range(_N_CORES)), trace=trace
    )
    out = np.concatenate([res.results[i]["y"] for i in range(_N_CORES)], axis=0)
    return out, res


def kernel(**inputs):
    out, _ = _run(inputs, trace=False)
    return out
